# revision 1
# baseline (speedup 1.0000x reference)
"""MeshConv GNN message-passing kernel for 8 TRN2 NeuronCores.

Reference computation (E=500000 edges, C=64 ch, OUT=128):
    n = x[clip(nb)]                          # (E, 4, C) gather
    feat = [x, min(n0,n1), max(n0,n1), min(n2,n3), max(n2,n3)]  # (E, 320)
    h = feat @ W.T                           # (E, 128)
    h = BatchNorm(h, training)  (global batch stats over E)
    out = relu(h)

Strategy: shard E across 8 cores (62500 edges each); x replicated in each
core's DRAM; neighbor rows fetched by indirect DMA (256B/row descriptors);
feat transposed to channel-major via PE transpose; GEMM with W.T chunks
stationary producing h in [out_ch, edge] layout; BN stats accumulated with
ACT accum_out and all-reduced across cores with a 1KB collective; phase B
applies the affine+ReLU per-partition and PE-transposes back to edge-major
for contiguous output writes.  h is held in SBUF (bf16) between phases.
"""

import numpy as np

import concourse.bass as bass
import concourse.bacc as bacc
import concourse.tile as tile
from concourse import mybir
from concourse.bass_utils import run_bass_kernel_spmd
from concourse.masks import make_identity

E, C, OUT = 500000, 64, 128
NCORES = 8
ES = E // NCORES            # 62500 edges per core
P = 128
GROUP = 512                 # edges per matmul group (PSUM bank = [128, 512] f32)
NSUB = GROUP // P           # 4 subtiles per group
NG = (ES + GROUP - 1) // GROUP   # 123 groups (122 full + 36-edge remainder)
ES_PAD = NG * GROUP         # 62976
NIDX = 5                    # self + 4 neighbors per edge
EPS = 1e-5

FP32 = mybir.dt.float32
BF16 = mybir.dt.bfloat16
INT32 = mybir.dt.int32


def _valid_edges(g, es=ES):
    return min(GROUP, es - g * GROUP)



def _copy(nc, use_scalar, out, in_):
    if use_scalar:
        nc.scalar.copy(out=out, in_=in_)
    else:
        nc.vector.tensor_copy(out=out, in_=in_)

def build_kernel(es=ES):
    ng = (es + GROUP - 1) // GROUP
    es_pad = ng * GROUP
    nc = bacc.Bacc("TRN2", num_devices=NCORES)

    x_t = nc.dram_tensor("x", [E, C], FP32, kind="ExternalInput")
    xself_t = nc.dram_tensor("xself", [es_pad, C], FP32, kind="ExternalInput")
    idx_t = nc.dram_tensor("idx", [P, ng * NSUB * NIDX], INT32, kind="ExternalInput")
    wt_t = nc.dram_tensor("wt", [3, P, OUT], FP32, kind="ExternalInput")
    gb_t = nc.dram_tensor("gb", [P, 2], FP32, kind="ExternalInput")
    out_t = nc.dram_tensor("out", [es, OUT], FP32, kind="ExternalOutput")

    cc_in = nc.dram_tensor("cc_in", [P, 2], FP32, kind="Internal")
    cc_out = nc.dram_tensor("cc_out", [P, 2], FP32, kind="Internal", addr_space="Shared")

    with tile.TileContext(nc) as tc:
        with (
            tc.tile_pool(name="singles", bufs=1) as singles,
            tc.tile_pool(name="stage", bufs=3) as stage,
            tc.tile_pool(name="featp", bufs=3) as featp,
            tc.tile_pool(name="featTp", bufs=3) as featTp,
            tc.tile_pool(name="hnp", bufs=3) as hnp,
            tc.tile_pool(name="obp", bufs=4) as obp,
            tc.tile_pool(name="psumT", bufs=4, space="PSUM") as psumT,
            tc.tile_pool(name="psumH", bufs=2, space="PSUM") as psumH,
            tc.tile_pool(name="psumB", bufs=2, space="PSUM") as psumB,
        ):
            # ---- constants / persistent state ----
            ident = singles.tile([P, P], FP32)
            make_identity(nc, ident[:])
            wt_sb = singles.tile([P, 3, OUT], FP32)
            nc.sync.dma_start(out=wt_sb[:], in_=wt_t[:, :, :].rearrange("c p o -> p c o"))
            gb_sb = singles.tile([P, 2], FP32)
            nc.sync.dma_start(out=gb_sb[:], in_=gb_t[:, :])
            idx_sb = singles.tile([P, ng * NSUB * NIDX], INT32)
            nc.sync.dma_start(out=idx_sb[:], in_=idx_t[:, :])

            h_sb = singles.tile([P, es_pad], BF16)
            s1parts = singles.tile([P, P], FP32)
            s2parts = singles.tile([P, P], FP32)
            nc.vector.memset(s1parts[:], 0.0)
            nc.vector.memset(s2parts[:], 0.0)

            # ---- phase A: gather -> feat -> transpose -> GEMM -> stats ----
            for g in range(ng):
                ne = _valid_edges(g, es)

                stg = stage.tile([P, NSUB, NIDX, C], FP32)
                # self rows: contiguous [j*P+p] -> stg[:, j, 0, :]
                nc.sync.dma_start(
                    out=stg[:, :, 0, :],
                    in_=xself_t[g * GROUP:(g + 1) * GROUP, :].rearrange(
                        "(a p) c -> p a c", p=P),
                )
                # neighbor rows: one indirect DMA per (subtile, slot)
                for j in range(NSUB):
                    for r in range(1, NIDX):
                        nc.gpsimd.indirect_dma_start(
                            out=stg[:, j, r, :],
                            out_offset=None,
                            in_=x_t[:, :],
                            in_offset=bass.IndirectOffsetOnAxis(
                                ap=idx_sb[:, (g * NSUB + j) * NIDX + r:
                                          (g * NSUB + j) * NIDX + r + 1],
                                axis=0,
                            ),
                        )

                # feat chunks 0/1 hold the pairwise min/max outputs:
                #   chunk0 = [p1_lo | p1_hi], chunk1 = [p2_lo | p2_hi]
                # chunk2 (self x, 64 ch) is read straight from stg.
                featc = featp.tile([P, 2, NSUB, P], FP32)
                nc.vector.tensor_tensor(
                    out=featc[:, 0, :, 0:C], in0=stg[:, :, 1, :], in1=stg[:, :, 2, :],
                    op=mybir.AluOpType.min)
                nc.vector.tensor_tensor(
                    out=featc[:, 0, :, C:2 * C], in0=stg[:, :, 1, :], in1=stg[:, :, 2, :],
                    op=mybir.AluOpType.max)
                nc.vector.tensor_tensor(
                    out=featc[:, 1, :, 0:C], in0=stg[:, :, 3, :], in1=stg[:, :, 4, :],
                    op=mybir.AluOpType.min)
                nc.vector.tensor_tensor(
                    out=featc[:, 1, :, C:2 * C], in0=stg[:, :, 3, :], in1=stg[:, :, 4, :],
                    op=mybir.AluOpType.max)

                # transpose feat to channel-major: featT[:, c, j, :] = feat_cj.T
                featT = featTp.tile([P, 3, NSUB, P], FP32)
                for j in range(NSUB):
                    for c in range(2):
                        pt = psumT.tile([P, P], FP32, tag="pt")
                        nc.tensor.transpose(
                            out=pt[:], in_=featc[:, c, j, :], identity=ident[:])
                        _copy(nc, (j + c) % 2 == 0, featT[:, c, j, :], pt[:])
                    pt = psumT.tile([P, P], FP32, tag="pt")
                    nc.tensor.transpose(
                        out=pt[0:C, :], in_=stg[:, j, 0, :], identity=ident[:])
                    _copy(nc, j % 2 == 0, featT[0:C, 2, j, :], pt[0:C, :])

                # GEMM: h[o, e] = sum_c WT[c, o] * featT[c, e]
                hp = psumH.tile([P, GROUP], FP32)
                nc.tensor.matmul(
                    out=hp[:], lhsT=wt_sb[:, 0, :], rhs=featT[:, 0, :, :],
                    start=True, stop=False)
                nc.tensor.matmul(
                    out=hp[:], lhsT=wt_sb[:, 1, :], rhs=featT[:, 1, :, :],
                    start=False, stop=False)
                nc.tensor.matmul(
                    out=hp[:], lhsT=wt_sb[0:C, 2, :], rhs=featT[0:C, 2, :, :],
                    start=False, stop=True)

                # store h (bf16) + accumulate per-channel sum / sum-of-squares
                nc.scalar.activation(
                    out=h_sb[:, g * GROUP:g * GROUP + ne], in_=hp[:, 0:ne],
                    func=mybir.ActivationFunctionType.Copy,
                    accum_out=s1parts[:, g:g + 1])
                hsq = stage.tile([P, GROUP], BF16, tag="hsq")
                nc.scalar.activation(
                    out=hsq[:, 0:ne], in_=hp[:, 0:ne],
                    func=mybir.ActivationFunctionType.Square,
                    accum_out=s2parts[:, g:g + 1])

            # ---- stats all-reduce + affine params ----
            S = singles.tile([P, 2], FP32)
            nc.vector.reduce_sum(out=S[:, 0:1], in_=s1parts[:], axis=mybir.AxisListType.X)
            nc.vector.reduce_sum(out=S[:, 1:2], in_=s2parts[:], axis=mybir.AxisListType.X)
            nc.sync.dma_start(out=cc_in[:, :], in_=S[:])
            nc.gpsimd.collective_compute(
                "AllReduce",
                mybir.AluOpType.add,
                ins=[cc_in[:, :]],
                outs=[cc_out[:, :]],
                replica_groups=[list(range(NCORES))],
            )
            Sg = singles.tile([P, 2], FP32)
            nc.sync.dma_start(out=Sg[:], in_=cc_out[:, :])

            prm = singles.tile([P, 6], FP32)
            mean, ex2, var, rstd, scl, bias = (prm[:, i:i + 1] for i in range(6))
            nc.scalar.mul(mean, Sg[:, 0:1], 1.0 / (es * NCORES))
            nc.scalar.mul(ex2, Sg[:, 1:2], 1.0 / (es * NCORES))
            nc.vector.tensor_tensor(out=var, in0=mean, in1=mean, op=mybir.AluOpType.mult)
            nc.vector.tensor_tensor(out=var, in0=ex2, in1=var, op=mybir.AluOpType.subtract)
            sd = singles.tile([P, 1], FP32)
            eps_sb = singles.tile([P, 1], FP32)
            nc.vector.memset(eps_sb[:], EPS)
            nc.scalar.activation(out=sd[:], in_=var, func=mybir.ActivationFunctionType.Sqrt,
                                 bias=eps_sb[:])
            nc.vector.reciprocal(out=rstd, in_=sd[:])
            nc.vector.tensor_tensor(out=scl, in0=gb_sb[:, 0:1], in1=rstd, op=mybir.AluOpType.mult)
            nc.vector.tensor_tensor(out=bias, in0=mean, in1=scl, op=mybir.AluOpType.mult)
            nc.vector.tensor_tensor(out=bias, in0=gb_sb[:, 1:2], in1=bias, op=mybir.AluOpType.subtract)

            # ---- phase B: affine+ReLU, transpose to edge-major, write out ----
            for g in range(ng):
                ne = _valid_edges(g, es)
                hn = hnp.tile([P, GROUP], FP32)
                nc.scalar.activation(
                    out=hn[:, 0:ne], in_=h_sb[:, g * GROUP:g * GROUP + ne],
                    func=mybir.ActivationFunctionType.Relu,
                    bias=bias, scale=scl)
                for j in range((ne + P - 1) // P):
                    nr = min(P, ne - j * P)
                    pb = psumB.tile([P, P], FP32, tag="pb")
                    nc.tensor.transpose(
                        out=pb[0:nr, :], in_=hn[:, j * P:j * P + nr], identity=ident[:])
                    ob = obp.tile([P, P], FP32)
                    _copy(nc, j % 2 == 0, ob[0:nr, :], pb[0:nr, :])
                    e0 = g * GROUP + j * P
                    nc.sync.dma_start(out=out_t[e0:e0 + nr, :], in_=ob[0:nr, :])

    nc.compile()
    return nc


def prep_inputs(x, nb, W, gamma, beta, es=ES):
    """Host-side layout prep: per-core index arrays, W.T chunks, gamma/beta pack."""
    idx = np.clip(nb, 0, E - 1).astype(np.int64)

    # WT rows reordered to feat order [p1_lo, p1_hi, p2_lo, p2_hi, x]:
    # chunks: 0 -> [W_p1lo; W_p1hi], 1 -> [W_p2lo; W_p2hi], 2 -> [W_x; pad]
    WT = np.ascontiguousarray(W.T.astype(np.float32))     # [320, 128]
    wt = np.zeros((3, P, OUT), dtype=np.float32)
    wt[0] = WT[C:3 * C]
    wt[1] = WT[3 * C:5 * C]
    wt[2, 0:C] = WT[0:C]

    gb = np.stack([gamma.astype(np.float32), beta.astype(np.float32)], axis=1)

    ng = (es + GROUP - 1) // GROUP
    es_pad = ng * GROUP
    in_maps = []
    for c in range(NCORES):
        base = c * ES
        sl = idx[base:base + es]                           # [es, 4]
        padded = np.zeros((es_pad, NIDX), dtype=np.int64)
        padded[:es, 0] = np.arange(base, base + es)        # self index
        padded[es:, 0] = base
        padded[:es, 1:] = sl
        padded[es:, 1:] = base
        # [g*GROUP + j*P + p, r] -> A[p, (g, j, r)]
        A = padded.reshape(ng, NSUB, P, NIDX).transpose(2, 0, 1, 3)
        A = np.ascontiguousarray(A.reshape(P, ng * NSUB * NIDX), dtype=np.int32)
        xs = np.zeros((es_pad, C), dtype=np.float32)
        xs[:es] = x[base:base + es]
        in_maps.append({
            "x": np.ascontiguousarray(x, dtype=np.float32),
            "xself": xs,
            "idx": A,
            "wt": wt,
            "gb": gb,
        })
    return in_maps


_NC_CACHE = {}


def kernel(x, nb, W, gamma, beta, _trace=False):
    x = np.asarray(x)
    nb = np.asarray(nb)
    W = np.asarray(W)
    gamma = np.asarray(gamma)
    beta = np.asarray(beta)

    if "nc" not in _NC_CACHE:
        _NC_CACHE["nc"] = build_kernel()
    nc = _NC_CACHE["nc"]

    in_maps = prep_inputs(x, nb, W, gamma, beta)
    res = run_bass_kernel_spmd(
        nc, in_maps, core_ids=list(range(NCORES)), trace=_trace,
    )
    out = np.concatenate([r["out"] for r in res.results], axis=0)
    _NC_CACHE["last_result"] = res
    return out



# revision 15
# speedup vs baseline: 1.6537x; 1.6537x over previous
"""MeshConv GNN message-passing kernel for 8 TRN2 NeuronCores.

Reference computation (E=500000 edges, C=64 ch, OUT=128):
    n = x[clip(nb)]                          # (E, 4, C) gather
    feat = [x, min(n0,n1), max(n0,n1), min(n2,n3), max(n2,n3)]  # (E, 320)
    h = feat @ W.T                           # (E, 128)
    h = BatchNorm(h, training)  (global batch stats over E)
    out = relu(h)

Strategy: shard E across 8 cores. The gather is the bottleneck: the device
ucode only supports 128-descriptor indirect DMAs (~1us each), so descriptor
COUNT is what matters. Each core gets its own bijective permutation of x
(xp = x[order]) built by a greedy packer that places each edge's 4 neighbor
rows consecutively (quad -> ONE 512B descriptor) or pairwise adjacent
(256B descriptors). Edges are sorted into homogeneous classes:
  Q  (quad packed):            4 indirect DMAs / 512-edge group
  PS (pair1 packed):          12 (4 pair + 8 single)
  SP (pair2 packed):          12
  SS (nothing packed):        16
Dummy edges (reading an appended all-zero row of xp) pad each class to
whole groups and to common per-class counts across cores, so the SPMD
program is uniform and BN stats stay exact (zero contributions; divisor is
the real edge count). Self features are loaded channel-major from a
host-pretransposed, edge-permuted copy; pairwise min/max fused into 2 wide
DVE ops per 4-group batch; feat chunks transposed via plain bf16 matmuls
against the identity into fp32 PSUM; 3 accumulating bf16 matmuls per
group; BN stats via scalar-engine accum (sum from PSUM h-copy, sum of
squares from bf16 h in SBUF); 1KB all-reduce; phase B applies
scale+bias+ReLU per-partition (channel-major) and writes the output
channel-major in bf16 -- the host inverts the edge permutation.
"""

import ml_dtypes
import numpy as np

import concourse.bass as bass
import concourse.bacc as bacc
import concourse.tile as tile
from concourse import mybir
from concourse.bass_utils import run_bass_kernel_spmd
from concourse.masks import make_identity

E, C, OUT = 500000, 64, 128
NCORES = 8
ES = E // NCORES            # 62500 edges per core
P = 128
GROUP = 512                 # edges per matmul group (PSUM bank = [128, 512] f32)
NSUB = GROUP // P           # 4 subtiles per group
BATCH = 4                   # groups per stage batch
EPS = 1e-5

FP32 = mybir.dt.float32
BF16 = mybir.dt.bfloat16
INT32 = mybir.dt.int32

BF = ml_dtypes.bfloat16

# idx columns per (group, j-subtile) for each class
CLS_COLS = {"Q": 1, "PS": 3, "SP": 3, "SS": 4}


def build_kernel(group_classes, real_total, ncores=NCORES):
    ng = len(group_classes)
    es_pad = ng * GROUP
    nbatch = (ng + BATCH - 1) // BATCH
    ncols = sum(CLS_COLS[c] * NSUB for c in group_classes)
    # column offset of each group's idx block
    col_of = []
    acc = 0
    for c in group_classes:
        col_of.append(acc)
        acc += CLS_COLS[c] * NSUB

    nc = bacc.Bacc("TRN2", num_devices=ncores)

    x_t = nc.dram_tensor("x", [E + 4, C], BF16, kind="ExternalInput")
    xt_t = nc.dram_tensor("xt", [C, es_pad], BF16, kind="ExternalInput")
    idx_t = nc.dram_tensor("idx", [P, ncols], INT32, kind="ExternalInput")
    wt_t = nc.dram_tensor("wt", [3, P, OUT], BF16, kind="ExternalInput")
    gb_t = nc.dram_tensor("gb", [P, 2], FP32, kind="ExternalInput")
    out_t = nc.dram_tensor("out", [OUT, es_pad], BF16, kind="ExternalOutput")

    if ncores > 1:
        cc_in = nc.dram_tensor("cc_in", [P, 2], FP32, kind="Internal")
        cc_out = nc.dram_tensor(
            "cc_out", [P, 2], FP32, kind="Internal", addr_space="Shared")

    with tile.TileContext(nc) as tc:
        with (
            tc.tile_pool(name="singles", bufs=1) as singles,
            tc.tile_pool(name="stage", bufs=3) as stage,
            tc.tile_pool(name="fcp", bufs=2) as fcp,
            tc.tile_pool(name="xTp", bufs=2) as xTp,
            tc.tile_pool(name="ftp", bufs=3) as ftp,
            tc.tile_pool(name="hsqp", bufs=2) as hsqp,
            tc.tile_pool(name="obp", bufs=4) as obp,
            tc.tile_pool(name="psumT", bufs=3, space="PSUM") as psumT,
            tc.tile_pool(name="psumH", bufs=2, space="PSUM") as psumH,
        ):
            # ---- constants / persistent state ----
            ident = singles.tile([P, P], BF16)
            make_identity(nc, ident[:])
            wt_sb = singles.tile([P, 3, OUT], BF16)
            nc.sync.dma_start(out=wt_sb[:], in_=wt_t[:, :, :].rearrange("c p o -> p c o"))
            gb_sb = singles.tile([P, 2], FP32)
            nc.sync.dma_start(out=gb_sb[:], in_=gb_t[:, :])
            idx_sb = singles.tile([P, ncols], INT32)
            nc.sync.dma_start(out=idx_sb[:], in_=idx_t[:, :])

            h_sb = singles.tile([P, es_pad], BF16)
            s1parts = singles.tile([P, ng], FP32)
            s2parts = singles.tile([P, nbatch], FP32)

            # ---- phase A (software-pipelined one group ahead) ----
            pend = None          # (g, ft, xT, g4) awaiting matmul+stats

            def emit_ttr(bb):
                off = bb * BATCH * GROUP
                valid_b = min(es_pad - off, BATCH * GROUP)
                hsq = hsqp.tile([P, BATCH * GROUP], BF16)
                nc.scalar.activation(
                    out=hsq[:, 0:valid_b], in_=h_sb[:, off:off + valid_b],
                    func=mybir.ActivationFunctionType.Square,
                    accum_out=s2parts[:, bb:bb + 1])

            def emit_pending():
                g, ft, pxT, g4 = pend
                hp = psumH.tile([P, GROUP], FP32)
                nc.tensor.matmul(
                    out=hp[:], lhsT=wt_sb[:, 0, :], rhs=ft[:, 0, :],
                    start=True, stop=False)
                nc.tensor.matmul(
                    out=hp[:], lhsT=wt_sb[:, 1, :], rhs=ft[:, 1, :],
                    start=False, stop=False)
                nc.tensor.matmul(
                    out=hp[:], lhsT=wt_sb[0:C, 2, :],
                    rhs=pxT[0:C, g4 * GROUP:g4 * GROUP + GROUP],
                    start=False, stop=True)
                # h -> SBUF bf16 with per-channel sum on ACT
                nc.scalar.activation(
                    out=h_sb[:, g * GROUP:(g + 1) * GROUP], in_=hp[:],
                    func=mybir.ActivationFunctionType.Copy,
                    accum_out=s1parts[:, g:g + 1])
                if g % BATCH == BATCH - 1 or g == ng - 1:
                    emit_ttr(g // BATCH)

            for b in range(nbatch):
                g0 = b * BATCH
                nga = min(BATCH, ng - g0)
                bcols = nga * GROUP

                xT = xTp.tile([C, BATCH * GROUP], BF16)
                nc.sync.dma_start(
                    out=xT[:, 0:bcols],
                    in_=xt_t[:, g0 * GROUP:g0 * GROUP + bcols],
                )
                # gather, slot order (n0, n1, n2, n3); instruction shape by
                # class: Q quad descs, PS/SP pair+singles, SS 4 singles
                stg = stage.tile([P, BATCH * NSUB, 4, C], BF16)
                for g4 in range(nga):
                    g = g0 + g4
                    cls = group_classes[g]
                    base_col = col_of[g]
                    for jg in range(NSUB):
                        ju = g4 * NSUB + jg
                        cb = base_col + jg * CLS_COLS[cls]

                        def gat(out_ap, col):
                            # the device ucode needs FLAT 2D out APs
                            nc.gpsimd.indirect_dma_start(
                                out=out_ap.rearrange("p a c -> p (a c)"),
                                out_offset=None,
                                in_=x_t[:, :],
                                in_offset=bass.IndirectOffsetOnAxis(
                                    ap=idx_sb[:, col:col + 1], axis=0),
                            )

                        if cls == "Q":
                            gat(stg[:, ju, :, :], cb)
                        elif cls == "PS":
                            gat(stg[:, ju, 0:2, :], cb)
                            gat(stg[:, ju, 2:3, :], cb + 1)
                            gat(stg[:, ju, 3:4, :], cb + 2)
                        elif cls == "SP":
                            gat(stg[:, ju, 0:1, :], cb)
                            gat(stg[:, ju, 1:2, :], cb + 1)
                            gat(stg[:, ju, 2:4, :], cb + 2)
                        else:
                            for r in range(4):
                                gat(stg[:, ju, r:r + 1, :], cb + r)

                # fc[:, g, 0, j, :] = [min(n0,n1) | min(n2,n3)]  (128 ch)
                # fc[:, g, 1, j, :] = [max(n0,n1) | max(n2,n3)]
                fc = fcp.tile([P, BATCH, 2, NSUB, P], BF16)
                nc.vector.tensor_tensor(
                    out=fc[:, 0:nga, 0, :, :],
                    in0=stg[:, 0:nga * NSUB, 0:4:2, :],
                    in1=stg[:, 0:nga * NSUB, 1:4:2, :],
                    op=mybir.AluOpType.min)
                nc.vector.tensor_tensor(
                    out=fc[:, 0:nga, 1, :, :],
                    in0=stg[:, 0:nga * NSUB, 0:4:2, :],
                    in1=stg[:, 0:nga * NSUB, 1:4:2, :],
                    op=mybir.AluOpType.max)

                for g4 in range(nga):
                    g = g0 + g4
                    # transpose via plain bf16 matmul against identity
                    # (out[m,n] = sum_k fc[k,m] I[k,n] = fc[n,m]); fp32 PSUM
                    ptl = psumT.tile([P, GROUP], FP32, tag="ptl")
                    pth = psumT.tile([P, GROUP], FP32, tag="pth")
                    for j in range(NSUB):
                        nc.tensor.matmul(
                            out=ptl[:, j * P:(j + 1) * P],
                            lhsT=fc[:, g4, 0, j, :], rhs=ident[:],
                            start=True, stop=True)
                        nc.tensor.matmul(
                            out=pth[:, j * P:(j + 1) * P],
                            lhsT=fc[:, g4, 1, j, :], rhs=ident[:],
                            start=True, stop=True)
                    ft = ftp.tile([P, 2, GROUP], BF16)
                    nc.vector.tensor_copy(out=ft[:, 0, :], in_=ptl[:])
                    nc.scalar.copy(out=ft[:, 1, :], in_=pth[:])

                    if pend is not None:
                        emit_pending()
                    pend = (g, ft, xT, g4)

            emit_pending()

            # ---- stats all-reduce + affine params ----
            S = singles.tile([P, 2], FP32)
            nc.vector.reduce_sum(out=S[:, 0:1], in_=s1parts[:], axis=mybir.AxisListType.X)
            nc.vector.reduce_sum(out=S[:, 1:2], in_=s2parts[:], axis=mybir.AxisListType.X)
            if ncores > 1:
                nc.sync.dma_start(out=cc_in[:, :], in_=S[:])
                nc.gpsimd.collective_compute(
                    "AllReduce",
                    mybir.AluOpType.add,
                    ins=[cc_in[:, :]],
                    outs=[cc_out[:, :]],
                    replica_groups=[list(range(ncores))],
                )
                Sg = singles.tile([P, 2], FP32)
                nc.sync.dma_start(out=Sg[:], in_=cc_out[:, :])
            else:
                Sg = S

            prm = singles.tile([P, 6], FP32)
            mean, ex2, var, rstd, scl, bias = (prm[:, i:i + 1] for i in range(6))
            nc.scalar.mul(mean, Sg[:, 0:1], 1.0 / real_total)
            nc.scalar.mul(ex2, Sg[:, 1:2], 1.0 / real_total)
            nc.vector.tensor_tensor(out=var, in0=mean, in1=mean, op=mybir.AluOpType.mult)
            nc.vector.tensor_tensor(out=var, in0=ex2, in1=var, op=mybir.AluOpType.subtract)
            sd = singles.tile([P, 1], FP32)
            eps_sb = singles.tile([P, 1], FP32)
            nc.vector.memset(eps_sb[:], EPS)
            nc.scalar.activation(out=sd[:], in_=var, func=mybir.ActivationFunctionType.Sqrt,
                                 bias=eps_sb[:])
            nc.vector.reciprocal(out=rstd, in_=sd[:])
            nc.vector.tensor_tensor(out=scl, in0=gb_sb[:, 0:1], in1=rstd, op=mybir.AluOpType.mult)
            nc.vector.tensor_tensor(out=bias, in0=mean, in1=scl, op=mybir.AluOpType.mult)
            nc.vector.tensor_tensor(out=bias, in0=gb_sb[:, 1:2], in1=bias, op=mybir.AluOpType.subtract)

            # ---- phase B: affine+ReLU in channel-major, write out ----
            OB = 2 * GROUP
            for u in range((es_pad + OB - 1) // OB):
                off = u * OB
                valid = min(es_pad - off, OB)
                ob = obp.tile([P, OB], BF16)
                nc.scalar.activation(
                    out=ob[:, 0:valid], in_=h_sb[:, off:off + valid],
                    func=mybir.ActivationFunctionType.Relu,
                    bias=bias, scale=scl)
                nc.sync.dma_start(out=out_t[:, off:off + valid], in_=ob[:, 0:valid])

    nc.compile()
    return nc


def _pack_core(idx):
    """Greedy adjacency packing for one core's [es, 4] neighbor indices.
    Returns (order, cls, pos): order is the xp row order (bijection over
    [0, E)), cls[e] in {0:Q, 1:PS, 2:SP, 3:SS}, pos[node] its xp position."""
    es = idx.shape[0]
    pos = np.full(E, -1, dtype=np.int64)
    order = np.empty(E, dtype=np.int64)
    nxt = 0
    cls = np.empty(es, dtype=np.int8)
    for e in range(es):
        a, b, c, d = idx[e]
        if a != b and a != c and a != d and b != c and b != d and c != d \
           and pos[a] < 0 and pos[b] < 0 and pos[c] < 0 and pos[d] < 0:
            order[nxt] = a; order[nxt + 1] = b
            order[nxt + 2] = c; order[nxt + 3] = d
            pos[a] = nxt; pos[b] = nxt + 1; pos[c] = nxt + 2; pos[d] = nxt + 3
            nxt += 4
            cls[e] = 0
            continue
        ok1 = a != b and pos[a] < 0 and pos[b] < 0
        if ok1:
            order[nxt] = a; order[nxt + 1] = b
            pos[a] = nxt; pos[b] = nxt + 1
            nxt += 2
        ok2 = c != d and pos[c] < 0 and pos[d] < 0
        if ok2:
            order[nxt] = c; order[nxt + 1] = d
            pos[c] = nxt; pos[d] = nxt + 1
            nxt += 2
        cls[e] = 1 if (ok1 and not ok2) else (2 if (ok2 and not ok1) else
                                              (1 if ok1 else 3))
    rest = np.where(pos < 0)[0]
    order[nxt:nxt + len(rest)] = rest
    pos[rest] = nxt + np.arange(len(rest))
    return order, cls, pos


def prep_inputs(x, nb, W, gamma, beta, es=ES, ncores=NCORES):
    x = np.asarray(x, dtype=np.float32)
    idx_all = np.clip(np.asarray(nb), 0, E - 1).astype(np.int64)

    WT = np.ascontiguousarray(np.asarray(W, np.float32).T)     # [320, 128]
    wt = np.zeros((3, P, OUT), np.float32)
    wt[0, 0:C] = WT[C:2 * C]          # p1_lo
    wt[0, C:2 * C] = WT[3 * C:4 * C]  # p2_lo
    wt[1, 0:C] = WT[2 * C:3 * C]      # p1_hi
    wt[1, C:2 * C] = WT[4 * C:5 * C]  # p2_hi
    wt[2, 0:C] = WT[0:C]              # x self
    wtbf = wt.astype(BF)
    gb = np.stack([np.asarray(gamma, np.float32),
                   np.asarray(beta, np.float32)], axis=1)

    packs = []
    counts = np.zeros((ncores, 4), np.int64)
    for c in range(ncores):
        base = c * ES
        order, cls, pos = _pack_core(idx_all[base:base + es])
        packs.append((order, cls, pos))
        for k in range(4):
            counts[c, k] = int((cls == k).sum())
    ngc = [int(-(-counts[:, k].max() // GROUP)) for k in range(4)]
    group_classes = (["Q"] * ngc[0] + ["PS"] * ngc[1] +
                     ["SP"] * ngc[2] + ["SS"] * ngc[3])
    ng = len(group_classes)
    es_pad = ng * GROUP
    ncols = sum(CLS_COLS[cc] * NSUB for cc in group_classes)

    in_maps = []
    metas = []
    for c in range(ncores):
        base = c * ES
        idx = idx_all[base:base + es]
        order, cls, pos = packs[c]
        xp = np.zeros((E + 4, C), np.float32)
        xp[:E] = x[order]
        # processing order: class-sorted real edges + per-class dummy pad
        perm_parts = []
        for k in range(4):
            ed = np.where(cls == k)[0]
            pad = ngc[k] * GROUP - len(ed)
            perm_parts.append(np.concatenate([ed, np.full(pad, -1, np.int64)]))
        perm = np.concatenate(perm_parts)           # [es_pad], -1 = dummy
        real = perm >= 0
        pe = perm[real]                              # real local edge ids

        # descriptor start positions per processed edge
        cols = np.full((es_pad, 4), E, np.int64)     # dummies -> zero row
        p_of = pos[idx]                              # [es, 4] positions
        k0 = 0
        for k, name in enumerate(("Q", "PS", "SP", "SS")):
            n = ngc[k] * GROUP
            seg = perm[k0:k0 + n]
            sreal = seg >= 0
            er = seg[sreal]
            blk = cols[k0:k0 + n]
            if name == "Q":
                blk[sreal, 0] = p_of[er, 0]
            elif name == "PS":
                blk[sreal, 0] = p_of[er, 0]
                blk[sreal, 1] = p_of[er, 2]
                blk[sreal, 2] = p_of[er, 3]
            elif name == "SP":
                blk[sreal, 0] = p_of[er, 0]
                blk[sreal, 1] = p_of[er, 1]
                blk[sreal, 2] = p_of[er, 2]
            else:
                for r in range(4):
                    blk[sreal, r] = p_of[er, r]
            k0 += n

        # pack into idx_sb layout [P, ncols]
        A = np.zeros((P, ncols), np.int32)
        colp = 0
        for g, name in enumerate(group_classes):
            w = CLS_COLS[name]
            blk = cols[g * GROUP:(g + 1) * GROUP, 0:w]      # [512, w]
            A[:, colp:colp + NSUB * w] = (
                blk.reshape(NSUB, P, w).transpose(1, 0, 2).reshape(P, NSUB * w))
            colp += NSUB * w

        # self features, permuted, channel-major; dummies zero
        xsT = np.zeros((C, es_pad), np.float32)
        xsT[:, real] = x[base + pe].T

        in_maps.append({
            "x": np.ascontiguousarray(xp.astype(BF)),
            "xt": np.ascontiguousarray(xsT.astype(BF)),
            "idx": A,
            "wt": wtbf,
            "gb": gb,
        })
        metas.append((real, pe))
    return in_maps, metas, group_classes


_NC_CACHE = {}


def kernel(x, nb, W, gamma, beta, _trace=False):
    x = np.asarray(x)
    nb = np.asarray(nb)
    W = np.asarray(W)
    gamma = np.asarray(gamma)
    beta = np.asarray(beta)

    in_maps, metas, group_classes = prep_inputs(x, nb, W, gamma, beta)
    key = tuple(group_classes)
    if _NC_CACHE.get("key") != key:
        _NC_CACHE["nc"] = build_kernel(group_classes, real_total=E)
        _NC_CACHE["key"] = key
    nc = _NC_CACHE["nc"]

    res = run_bass_kernel_spmd(
        nc, in_maps, core_ids=list(range(NCORES)), trace=_trace,
    )
    out = np.empty((E, OUT), np.float32)
    for c in range(NCORES):
        arr = np.asarray(res.results[c]["out"]).T.astype(np.float32)
        real, pe = metas[c]
        out[c * ES + pe] = arr[real]
    _NC_CACHE["last_result"] = res
    return out
